# revision 17
# baseline (speedup 1.0000x reference)
"""Trainium2 Bass kernel for nn_MAB (dense transformer attention block).

Reference computation (fp32, single-device):
  q = Q @ Wq.T + bq ; k = K @ Wk.T + bk ; v = K @ Wv.T + bv     [2048, 1024]
  split into H=16 heads of d=64 (head h = contiguous 64-col slice)
  A = softmax(Q_ @ K_^T) / sqrt(1024)  per head                 [16, 2048, 2048]
  O = (Q_ + A @ V_) reshaped back                               [2048, 1024]
  out = O + relu(O @ Wo.T + bo)

Sharding: tensor-parallel over the 16 heads -> 2 heads (128 feature cols)
per NeuronCore, 8 cores. Everything on-core runs in TRANSPOSED layout
(features on partitions, tokens on the free axis), which makes all biases
per-partition and the attention softmax expressible without any
partition-axis reductions:

  - scores are built transposed:  S^T = k_h @ q_h^T    [n_k, n_q]
  - exp via ScalarE (no max subtraction needed: |S| <~ 20)
  - row sums of exp ride along the A@V matmul as a ones-column of V_aug:
      [U^T ; r] = [v_h | 1]^T @ E^T
  - 1/(32 r) broadcast across partitions via a K=1 matmul with a 1/32
    constant row, then fused into the output on VectorE.

Reference quirk: `O = (Q_ + A@V).reshape(-1, DIM_V)` flattens the
[16, 2048, 64] head-major tensor straight into [2048, 1024] WITHOUT
transposing heads back: O[h*128 + m, 64t + d] = Ohead_h[16m + t, d].
Every output row therefore depends on exactly one head, the final
projection is row-independent, and NO collective is needed: core c
(heads 2c, 2c+1) owns output rows [256c, 256(c+1)).

Final stage per head: Z = Oresh @ Wo.T built with the scrambled-column
blocks read straight out of O_att^T via stride-16 APs (block beta of 64
c-columns = oatt[h-slice, beta::16]); bias via a K=1 ones-row matmul;
residual from a PE-transpose + HBM-roundtrip reshape of O_att^T.

matmul dtypes: bf16 for the in-projections / A@V / out-projection
(inputs pre-cast on host), float32r (full-rate fp32) for the scores.
"""

import numpy as np
import ml_dtypes

import concourse.bass as bass
import concourse.tile as tile
from concourse import bacc, mybir
from concourse import bass_utils
from concourse.masks import make_identity

F32 = mybir.dt.float32
F32R = mybir.dt.float32r
BF16 = mybir.dt.bfloat16
AF = mybir.ActivationFunctionType
ALU = mybir.AluOpType

N = 2048          # tokens (n_q == n_k)
D = 1024          # model dim
NCORES = 8
FS = 128          # feature columns per core
NH = 2            # heads per core
HD = 64           # head dim
KK = D // 128     # contraction tiles over model dim
TK = N // 128     # token tiles
BF = ml_dtypes.bfloat16

_CACHED_NC = None
DEBUG = False


def build_program():
    nc = bacc.Bacc("TRN2", target_bir_lowering=False, debug=False,
                   enable_asserts=False, num_devices=NCORES)

    qt_d = nc.dram_tensor("qt", [D, N], BF16, kind="ExternalInput").ap()
    kt_d = nc.dram_tensor("kt", [D, N], BF16, kind="ExternalInput").ap()
    wqt_d = nc.dram_tensor("wqt", [KK, 128, FS], BF16, kind="ExternalInput").ap()
    wkt_d = nc.dram_tensor("wkt", [KK, 128, FS], BF16, kind="ExternalInput").ap()
    wvt_d = nc.dram_tensor("wvt", [KK, 128, FS], BF16, kind="ExternalInput").ap()
    wotdup_d = nc.dram_tensor("wotdup", [128, 16, D], BF16, kind="ExternalInput").ap()
    bq_d = nc.dram_tensor("bq", [FS, 1], F32, kind="ExternalInput").ap()
    bk_d = nc.dram_tensor("bk", [FS, 1], F32, kind="ExternalInput").ap()
    bv_d = nc.dram_tensor("bv", [FS, 1], F32, kind="ExternalInput").ap()
    bor_d = nc.dram_tensor("bor", [1, D], BF16, kind="ExternalInput").ap()
    out_d = nc.dram_tensor("out_rows", [NH * 128, D], F32, kind="ExternalOutput").ap()
    if DEBUG:
        dbg_qt = nc.dram_tensor("dbg_qt", [FS, N], F32R, kind="ExternalOutput").ap()
        dbg_kt = nc.dram_tensor("dbg_kt", [FS, N], F32R, kind="ExternalOutput").ap()
        dbg_oatt = nc.dram_tensor("dbg_oatt", [FS, N], F32, kind="ExternalOutput").ap()
        dbg_vn = nc.dram_tensor("dbg_vn", [128, NH, TK, HD + 1], BF16, kind="ExternalOutput").ap()
        dbg_e = nc.dram_tensor("dbg_e", [128, TK, NH, N], BF16, kind="ExternalOutput").ap()

    with tile.TileContext(nc) as tc:
        with tc.tile_pool(name="persist", bufs=1) as persist, \
             tc.tile_pool(name="consts", bufs=1) as consts, \
             tc.tile_pool(name="dram", bufs=1, space="DRAM") as dram:

            w_q = consts.tile([128, KK, FS], BF16)
            w_k = consts.tile([128, KK, FS], BF16)
            w_v = consts.tile([128, KK, FS], BF16)
            for kk in range(KK):
                nc.sync.dma_start(w_q[:, kk, :], wqt_d[kk])
                nc.sync.dma_start(w_k[:, kk, :], wkt_d[kk])
                nc.sync.dma_start(w_v[:, kk, :], wvt_d[kk])
            b_q = consts.tile([FS, 1], F32)
            nc.sync.dma_start(b_q[:], bq_d[:])
            b_k = consts.tile([FS, 1], F32)
            nc.sync.dma_start(b_k[:], bk_d[:])
            b_v = consts.tile([FS, 1], F32)
            nc.sync.dma_start(b_v[:], bv_d[:])
            # full-height constants so sliced rows share the operand's
            # base partition (walrus checkSBSameStartPartition)
            ones32f = consts.tile([128, HD], F32)
            nc.vector.memset(ones32f[:], 1.0 / 32.0)
            ones32 = consts.tile([128, HD], F32R)
            nc.vector.tensor_copy(ones32[:], ones32f[:])
            # identity replicated in both 64-partition halves so the
            # per-head transpose operands share a base partition
            ident = consts.tile([128, HD], F32)
            make_identity(nc, ident[0:HD, :])
            make_identity(nc, ident[HD:128, :])

            qt_s = persist.tile([FS, N], F32R)  # projected q, transposed (f32r for PE)
            kt_s = persist.tile([FS, N], F32R)
            oatt = persist.tile([FS, N], F32)   # attention output, transposed
            vnat = persist.tile([128, NH, TK, HD + 1], BF16)  # [v | 1] per head
            nc.vector.memset(vnat[:, :, :, HD:HD + 1], 1.0)

            # ---------------- Phase 1: projections (transposed) ------------
            with tc.tile_pool(name="p1io", bufs=4) as p1io, \
                 tc.tile_pool(name="p1tmp", bufs=1) as p1tmp, \
                 tc.tile_pool(name="p1ps", bufs=1, space="PSUM") as p1ps, \
                 tc.tile_pool(name="p1pst", bufs=2, space="PSUM") as p1pst:
                vt_s = p1tmp.tile([FS, N], F32)
                for half in range(2):
                    cs = slice(half * 1024, (half + 1) * 1024)
                    ps_q = p1ps.tile([128, 2, 512], F32, tag="psq")
                    ps_k = p1ps.tile([128, 2, 512], F32, tag="psk")
                    ps_v = p1ps.tile([128, 2, 512], F32, tag="psv")
                    for kk in range(KK):
                        qsl = p1io.tile([128, 1024], BF16, tag="qsl")
                        nc.sync.dma_start(qsl[:], qt_d[kk * 128:(kk + 1) * 128, cs])
                        ksl = p1io.tile([128, 1024], BF16, tag="ksl")
                        nc.sync.dma_start(ksl[:], kt_d[kk * 128:(kk + 1) * 128, cs])
                        st = dict(start=(kk == 0), stop=(kk == KK - 1))
                        for sub in range(2):
                            ss = slice(sub * 512, (sub + 1) * 512)
                            nc.tensor.matmul(ps_q[:, sub, :], w_q[:, kk, :], qsl[:, ss], **st)
                            nc.tensor.matmul(ps_k[:, sub, :], w_k[:, kk, :], ksl[:, ss], **st)
                            nc.tensor.matmul(ps_v[:, sub, :], w_v[:, kk, :], ksl[:, ss], **st)
                    nc.vector.tensor_scalar_add(qt_s[:, cs], ps_q[:, :, :].rearrange("p a b -> p (a b)"), b_q[:])
                    nc.vector.tensor_scalar_add(kt_s[:, cs], ps_k[:, :, :].rearrange("p a b -> p (a b)"), b_k[:])
                    nc.vector.tensor_scalar_add(vt_s[:, cs], ps_v[:, :, :].rearrange("p a b -> p (a b)"), b_v[:])
                # transpose v to natural layout (per head, per token tile)
                for h in range(NH):
                    for t in range(TK):
                        pt = p1pst.tile([128, HD], F32, tag="pt")
                        nc.tensor.transpose(
                            pt[:], vt_s[h * HD:(h + 1) * HD, t * 128:(t + 1) * 128],
                            ident[h * HD:(h + 1) * HD, :])
                        nc.vector.tensor_copy(vnat[:, h, t, 0:HD], pt[:])

            # ---------------- Phase 2: attention ---------------------------
            with tc.tile_pool(name="ep", bufs=1) as ep, \
                 tc.tile_pool(name="p2ps", bufs=2, space="PSUM") as p2ps, \
                 tc.tile_pool(name="p2av", bufs=2, space="PSUM") as p2av, \
                 tc.tile_pool(name="p2bc", bufs=2, space="PSUM") as p2bc, \
                 tc.tile_pool(name="p2sb", bufs=3) as p2sb:
                e_t = ep.tile([128, TK, NH, N], BF16)  # exp(S^T), both heads
                # scores + exp; the two heads' K=64 matmuls go to row groups
                # 0 and 64 and run concurrently in the PE array.
                for t in range(TK):
                    ts_ = slice(t * 128, (t + 1) * 128)
                    for ch in range(4):
                        cs = slice(ch * 512, (ch + 1) * 512)
                        ps = p2ps.tile([128, 2, 512], F32, tag="scores")
                        nc.tensor.matmul(ps[:, 0, :],
                                         kt_s[0:HD, ts_],
                                         qt_s[0:HD, cs],
                                         start=True, stop=True)
                        nc.tensor.matmul(ps[:, 1, :],
                                         kt_s[HD:128, ts_],
                                         qt_s[HD:128, cs],
                                         start=True, stop=True)
                        nc.scalar.activation(e_t[:, t, :, cs], ps[:, :, :], AF.Exp)
                # A@V with fused row-sums; scale + residual epilogue
                for h in range(NH):
                    hs = slice(h * HD, (h + 1) * HD)
                    for ch in range(4):
                        cs = slice(ch * 512, (ch + 1) * 512)
                        pu = p2av.tile([128, 512], F32, tag="pu")
                        for t in range(TK):
                            nc.tensor.matmul(pu[0:HD + 1, :], vnat[:, h, t, :],
                                             e_t[:, t, h, cs],
                                             start=(t == 0), stop=(t == TK - 1))
                        rinv = p2sb.tile([128, 512], F32R, tag="rinv")
                        with nc.allow_low_precision(reason="softmax scale in f32r"):
                            nc.vector.reciprocal(rinv[HD:HD + 1, :], pu[HD:HD + 1, :])
                        pb = p2bc.tile([HD, 512], F32, tag="pb")
                        nc.tensor.matmul(pb[:], ones32[HD:HD + 1, :],
                                         rinv[HD:HD + 1, :],
                                         start=True, stop=True)
                        sb = p2sb.tile([HD, 512], F32, tag="sb")
                        nc.vector.tensor_copy(sb[:], pb[:])
                        tmp = p2sb.tile([HD, 512], F32, tag="tmp")
                        nc.vector.tensor_mul(tmp[:], pu[0:HD, :], sb[:])
                        if h == 0:
                            nc.vector.tensor_add(oatt[hs, cs], tmp[:],
                                                 qt_s[hs, cs].bitcast(F32))
                        else:
                            # DVE copies may shift partition base (matmuls and
                            # tensor_tensor may not): scale at base 0, copy the
                            # result up to partitions 64..127, add residual there.
                            stage = p2sb.tile([128, 512], F32, tag="stage")
                            nc.vector.tensor_copy(stage[HD:128, :], tmp[:])
                            nc.vector.tensor_add(oatt[hs, cs], stage[HD:128, :],
                                                 qt_s[hs, cs].bitcast(F32))

                if DEBUG:
                    nc.sync.dma_start(dbg_qt[:], qt_s[:])
                    nc.sync.dma_start(dbg_kt[:], kt_s[:])
                    nc.sync.dma_start(dbg_oatt[:], oatt[:])
                    nc.sync.dma_start(dbg_vn[:], vnat[:])
                    for t in range(TK):
                        nc.sync.dma_start(dbg_e[:, t, :, :], e_t[:, t, :, :])

            # ------- Phase 3/4: scrambled-layout output projection ----------
            # O[h*128+m, 64t+d] = Ohead[16m+t, d]; rows are head-local.
            with tc.tile_pool(name="p4sb", bufs=2) as p4sb, \
                 tc.tile_pool(name="p4c", bufs=1) as p4c, \
                 tc.tile_pool(name="p4ps", bufs=2, space="PSUM") as p4ps, \
                 tc.tile_pool(name="p4pst", bufs=2, space="PSUM") as p4pst:
                # bf16 copy of O_att^T for the TensorE side
                oattbf = p4c.tile([FS, N], BF16)
                nc.vector.tensor_copy(oattbf[:], oatt[:])
                # residual reshape: PE-transpose O_att^T to natural token-major
                # tiles, spill to HBM, read back reshaped [128 m, 1024 (t d)]
                ohn = dram.tile([NH, N, HD], F32)
                for h in range(NH):
                    for t in range(TK):
                        pt2 = p4pst.tile([128, HD], F32, tag="pt2")
                        nc.tensor.transpose(
                            pt2[:], oatt[h * HD:(h + 1) * HD, t * 128:(t + 1) * 128],
                            ident[h * HD:(h + 1) * HD, :])
                        st2 = p4sb.tile([128, HD], F32, tag="st2")
                        nc.vector.tensor_copy(st2[:], pt2[:])
                        nc.sync.dma_start(ohn[h, t * 128:(t + 1) * 128, :], st2[:])
                # weights: WoT 64-row blocks duplicated into both partition
                # halves so the two heads' K=64 matmuls pack row groups 0/64
                wdup = p4c.tile([128, 16, D], BF16)
                for b in range(16):
                    nc.sync.dma_start(wdup[:, b, :], wotdup_d[:, b, :])
                onesb = p4c.tile([1, 128], BF16)
                nc.vector.memset(onesb[:], 1.0)
                bor = p4c.tile([1, D], BF16)
                nc.sync.dma_start(bor[:], bor_d[:])

                for h in range(NH):
                    hb = slice(h * HD, (h + 1) * HD)
                    oview = oattbf[hb, :].rearrange("d (m t) -> d t m", t=16)
                    ores = p4sb.tile([128, D], F32, tag="ores")
                    nc.sync.dma_start(ores[:], ohn[h].rearrange("(m t) d -> m (t d)", t=16))
                    zps = p4ps.tile([128, 2, 512], F32, tag="zps")
                    for jc in range(2):
                        js = slice(jc * 512, (jc + 1) * 512)
                        # bias first (start=True clears), then accumulate blocks
                        nc.tensor.matmul(zps[:, jc, :], onesb[:, :], bor[:, js],
                                         start=True, stop=False)
                        for b in range(16):
                            nc.tensor.matmul(zps[:, jc, :], oview[:, b, :],
                                             wdup[hb, b, js],
                                             start=False, stop=(b == 15))
                    for jc in range(2):
                        js = slice(jc * 512, (jc + 1) * 512)
                        rl = p4sb.tile([128, 512], F32, tag="rl")
                        nc.vector.tensor_scalar_max(rl[:], zps[:, jc, :], 0.0)
                        osb = p4sb.tile([128, 512], F32, tag="osb")
                        nc.vector.tensor_add(osb[:], rl[:], ores[:, js])
                        nc.sync.dma_start(out_d[h * 128:(h + 1) * 128, js], osb[:])

    nc.compile()
    return nc


def _prep_inputs(Q, K, Wq, bq, Wk, bk, Wv, bv, Wo, bo):
    qt = np.ascontiguousarray(Q.T).astype(BF)
    kt = np.ascontiguousarray(K.T).astype(BF)
    blocks = np.ascontiguousarray(Wo.T).reshape(16, 64, D).transpose(1, 0, 2)
    wotdup = np.ascontiguousarray(np.concatenate([blocks, blocks], axis=0)).astype(BF)
    bor_row = np.ascontiguousarray(bo.reshape(1, D)).astype(BF)
    in_maps = []
    for c in range(NCORES):
        fs = slice(c * FS, (c + 1) * FS)
        m = {
            "qt": qt,
            "kt": kt,
            "wqt": np.ascontiguousarray(Wq[fs, :].T).astype(BF).reshape(KK, 128, FS),
            "wkt": np.ascontiguousarray(Wk[fs, :].T).astype(BF).reshape(KK, 128, FS),
            "wvt": np.ascontiguousarray(Wv[fs, :].T).astype(BF).reshape(KK, 128, FS),
            "bq": np.ascontiguousarray(bq[fs].reshape(FS, 1)).astype(np.float32),
            "bk": np.ascontiguousarray(bk[fs].reshape(FS, 1)).astype(np.float32),
            "bv": np.ascontiguousarray(bv[fs].reshape(FS, 1)).astype(np.float32),
            "wotdup": wotdup,
            "bor": bor_row,
        }
        in_maps.append(m)
    return in_maps


def kernel(Q, K, Wq, bq, Wk, bk, Wv, bv, Wo, bo):
    global _CACHED_NC
    if _CACHED_NC is None:
        _CACHED_NC = build_program()
    nc = _CACHED_NC
    in_maps = _prep_inputs(Q, K, Wq, bq, Wk, bk, Wv, bv, Wo, bo)
    res = bass_utils.run_bass_kernel_spmd(
        nc, in_maps, core_ids=list(range(NCORES)), trace=False)
    out = np.empty((N, D), dtype=np.float32)
    for c in range(NCORES):
        out[c * 256:(c + 1) * 256, :] = res.results[c]["out_rows"]
    return out
